# revision 1
# baseline (speedup 1.0000x reference)
"""ClassBalancedSupConLoss on 8 TRN2 NeuronCores (Bass/Tile).

Math (reference semantics, reorganized for hardware):
  - All embeddings are unit-norm, so s_ij = e_i . e_j <= 1 and s_ii ~= 1.
    Use a FIXED logsumexp shift m = 1:
        LSE_i = inv_t_i * 1 + log( sum_j exp(inv_t_i * (s_ij - 1)) )
    The self term is excluded by subtracting exp(inv_t*(s_ii-1)) where
    s_ii is computed ON DEVICE from the same rounded operands (bitwise
    identical to the self term inside the big sum, so the cancellation
    is exact even though matmul-input rounding makes s_ii != 1).
  - Batch and bank are sorted by class on the host, so the same-class
    column set of any anchor is one contiguous segment.  Bank same-class
    exclusion = (total exp sum) - (own-class segment exp sum); positives
    = (own-class raw-logit segment sum - s_ii) / pos_cnt.
  - Anchors (batch rows) are sharded 256/core across 8 cores; every core
    holds full embT/bankT replicas.  Per-anchor losses are DMA'd out;
    the final masked mean over 2048 anchors is a host-side reduction.

Engine structure per core (2 anchor tiles x [128 anchors]):
  - PE: S chunks [128, 512] into rotating [128, 2048] PSUM tiles
    (2 tiles x 4 banks).  bf16 inputs (fast FWL weight loads, 1 cyc/row).
  - ACT: one Exp pass per 2048-col PSUM chunk with accum_out row-sums;
    exp calls are SPLIT at class-segment boundaries, so per-class bank
    exp sums fall out of the per-call accumulators directly.
  - DVE: raw-logit segment reductions for positives + tiny epilogue.

SPMD: one program for all 8 cores.  Anything core-dependent (the anchor
slice, per-anchor temperature vectors, one-hot class rows) is passed as
per-core DATA; program constants (class segment boundaries) are global.
"""

import os
import numpy as np

import concourse.bass as bass  # noqa: F401
from concourse import bacc
import concourse.mybir as mybir
import concourse.tile as tile
from concourse.bass_utils import run_bass_kernel_spmd

B, D, M, C = 2048, 128, 16384, 3
NCORES = 8
APC = B // NCORES          # anchors per core = 256
NT = APC // 128            # anchor tiles per core = 2
CH = 512                   # matmul free chunk (one PSUM bank)
W = 2048                   # big PSUM chunk (4 banks) = one ACT Exp pass
NBK = M // W               # 8 bank pieces of [128, 2048]
BASE_TEMP = 0.07

F32 = mybir.dt.float32
AF = mybir.ActivationFunctionType
ALU = mybir.AluOpType
AX = mybir.AxisListType

# "bf16": matmul inputs bf16 (fast path; ~1e-3 logit rounding)
# "f32r": fp32 bits, PE rounds mantissa (slow LDWEIGHTS, ~4x PE time)
# "f32" : full fp32 matmul (4 cyc/row)
MM_MODE = os.environ.get("SUPCON_MM_MODE", "bf16")

LAST_EXEC_TIME_NS = None   # set by kernel() when SUPCON_TRACE=1


def _install_trace_shim():
    """Register the NTFF profile hook that this image's antenv lacks.

    Mirrors trn_agent_boot's _ntff_profile_via_ctypes: drives NRT
    profiling via the injected libaxon_pjrt.so.  Only used for local
    perf iteration (SUPCON_TRACE=1); the plain execution path never
    needs it.
    """
    import sys
    import types
    import ctypes
    import contextlib

    try:
        from antenv.axon_hooks import get_axon_ntff_profile_hook  # noqa: F401
        return True  # real module exists
    except ImportError:
        pass

    so_path = "/opt/axon/libaxon_pjrt.so"
    if not os.path.exists(so_path):
        return False
    lib = ctypes.CDLL(so_path)
    if not hasattr(lib, "axon_start_nrt_profile"):
        return False
    lib.axon_start_nrt_profile.argtypes = [
        ctypes.POINTER(ctypes.c_int64),
        ctypes.c_size_t,
    ]
    lib.axon_start_nrt_profile.restype = ctypes.c_int64
    lib.axon_stop_nrt_profile.argtypes = [ctypes.c_char_p]
    lib.axon_stop_nrt_profile.restype = ctypes.c_int64

    @contextlib.contextmanager
    def _hook(output_dir, device_ids):
        import jax

        jax.devices()
        if device_ids:
            ids = (ctypes.c_int64 * len(device_ids))(*device_ids)
            rc = lib.axon_start_nrt_profile(ids, len(device_ids))
        else:
            rc = lib.axon_start_nrt_profile(None, 0)
        if rc != 0:
            raise RuntimeError(f"axon_start_nrt_profile rc={rc}")
        try:
            yield
        finally:
            n = lib.axon_stop_nrt_profile(str(output_dir).encode())
            print(f"profile: {n} file(s) written to {output_dir}", file=sys.stderr)

    _state = {"hook": _hook}
    mod = types.ModuleType("antenv.axon_hooks")
    mod.get_axon_ntff_profile_hook = lambda: _state["hook"]
    mod.set_axon_ntff_profile_hook = lambda h: _state.update(hook=h)
    sys.modules["antenv.axon_hooks"] = mod
    import antenv

    antenv.axon_hooks = mod

    # skip the artifact upload (no bucket access needed for local iteration)
    import concourse.bass_utils as bu

    bu.upload_artifacts = lambda tmpdir: tmpdir
    return True


def _bank_subranges(mk_b1, mk_b2):
    """Split [0, M) at big-chunk multiples AND class boundaries.

    Returns (subs, i1, i2): subs = list of (start, end); i1/i2 = first
    subrange index at/after mk_b1/mk_b2 (class-segment column ranges in
    the per-subrange accumulator tile are then [0,i1), [i1,i2), [i2,n)).
    """
    cuts = sorted({c * W for c in range(NBK + 1)} | {mk_b1, mk_b2})
    subs = [(cuts[i], cuts[i + 1]) for i in range(len(cuts) - 1)]
    i1 = sum(1 for s, _ in subs if s < mk_b1)
    i2 = sum(1 for s, _ in subs if s < mk_b2)
    return subs, i1, i2


def _build(bb_b1, bb_b2, mk_b1, mk_b2, mm_mode):
    import ml_dtypes  # noqa: F401  (bf16 numpy dtype registration)

    if mm_mode == "bf16":
        in_dt = mybir.dt.bfloat16
    elif mm_mode == "f32":
        in_dt = F32
    else:
        in_dt = mybir.dt.float32r

    nc = bacc.Bacc()
    embT_d = nc.declare_dram_parameter("embT", [D, B], in_dt, isOutput=False)
    anchT_d = nc.declare_dram_parameter("anchT", [D, APC + C], in_dt, isOutput=False)
    bankT_d = nc.declare_dram_parameter("bankT", [D, M], in_dt, isOutput=False)
    subs, i1, i2 = _bank_subranges(mk_b1, mk_b2)
    NK = len(subs)
    # one packed small-vector input: [invt | ninvt | invpc | coefv | oneh |
    # incl | eye] along columns -- a single DMA instead of seven
    NV = NT * (4 + C + NK) + 128
    vecs_d = nc.declare_dram_parameter("vecs", [128, NV], F32, isOutput=False)
    oout_d = nc.declare_dram_parameter("oout", [128, 2 * NT], F32, isOutput=True)

    with tile.TileContext(nc) as tc:
        with (
            tc.tile_pool(name="big", bufs=1) as bigp,
            tc.tile_pool(name="sm", bufs=1) as smp,
            tc.tile_pool(name="ps", bufs=2, space="PSUM") as psp,
        ):
            anch_t = bigp.tile([D, APC + C], in_dt, tag="anchT")
            vecs_t = smp.tile([128, NV], F32, tag="vecs")
            # garbage-operand warmup tiles (never written: no DMA dependency,
            # so the PE can start immediately and open the HAM clock gate)
            junkw_t = bigp.tile([128, 128], in_dt, tag="junkw")
            junkx_t = bigp.tile([128, CH], in_dt, tag="junkx")
            o = [0]
            def vslice(w):
                a = o[0]; o[0] += w
                return vecs_t[:, a:a + w]
            invt_t = vslice(NT)
            ninvt_t = vslice(NT)
            invpc_t = vslice(NT)
            coefv_t = vslice(NT)
            oneh_t = vslice(NT * C)
            incl_t = vslice(NT * NK)
            eye_t = vslice(128)
            # both HWDGE queues (sync + scalar), pieces ordered by the time
            # the chunk stream consumes them; vecs first (unblocks the ACT
            # warmup), emb at quarter grain so the first bb matmuls start
            # as soon as the first 512 columns land
            emb_t = bigp.tile([D, B], in_dt, tag="embT")
            bank_ts = [bigp.tile([D, W], in_dt, tag=f"bank{j}", name=f"bank{j}")
                       for j in range(NBK)]
            H = B // 2
            Q = B // 4
            nc.sync.dma_start(out=vecs_t[:], in_=vecs_d[:])
            nc.scalar.dma_start(out=anch_t[:], in_=anchT_d[:])
            nc.sync.dma_start(out=emb_t[:, 0:Q], in_=embT_d[:, 0:Q])
            nc.scalar.dma_start(out=emb_t[:, Q:H], in_=embT_d[:, Q:H])
            nc.sync.dma_start(out=emb_t[:, H:H + Q], in_=embT_d[:, H:H + Q])
            nc.scalar.dma_start(out=emb_t[:, H + Q:B], in_=embT_d[:, H + Q:B])
            nc.sync.dma_start(out=bank_ts[0][:, 0:H], in_=bankT_d[:, 0:H])
            nc.scalar.dma_start(out=bank_ts[0][:, H:W], in_=bankT_d[:, H:W])
            nc.sync.dma_start(out=bank_ts[1][:, 0:H], in_=bankT_d[:, W:W + H])
            nc.scalar.dma_start(out=bank_ts[1][:, H:W], in_=bankT_d[:, W + H:2 * W])
            for j in range(2, NBK):
                eng = nc.sync if j % 2 == 0 else nc.scalar
                eng.dma_start(out=bank_ts[j][:], in_=bankT_d[:, j * W:(j + 1) * W])

            oout_t = smp.tile([128, 2 * NT], F32, tag="oout")
            scr_t = smp.tile([128, W], F32, tag="scrshared")
            sdiag = [smp.tile([128, 1], F32, tag=f"sdiag{t}", name=f"sdiag{t}") for t in range(NT)]
            selfe = [smp.tile([128, 1], F32, tag=f"selfe{t}", name=f"selfe{t}") for t in range(NT)]
            eyemul = smp.tile([128, 128], F32, tag="eyemul")
            warm = smp.tile([128, 1], F32, tag="warm")
            bbsum = [smp.tile([128, 1], F32, tag=f"bbsum{t}", name=f"bbsum{t}") for t in range(NT)]
            raw3 = [smp.tile([128, C], F32, tag=f"raw3{t}", name=f"raw3{t}") for t in range(NT)]
            esum = [smp.tile([128, NK], F32, tag=f"esum{t}", name=f"esum{t}") for t in range(NT)]

            # pull the Exp table load off the critical path
            nc.scalar.activation(warm[:], eye_t[:, 0:1], AF.Exp)

            def anch(t):
                return anch_t[:, t * 128:(t + 1) * 128]

            # ~4.3us of contiguous PE activity before the DMAs land: HAM
            # un-throttles (1.2 -> 2.4 GHz) before the real stream begins
            nc.vector.memset(junkw_t[:], 0.0)
            nc.vector.memset(junkx_t[:], 0.0)
            warm_ps = psp.tile([128, W], F32, tag="chunk", name="warm_ps")
            for w in range(8):
                nc.tensor.matmul(
                    warm_ps[:, (w % 4) * CH:((w % 4) + 1) * CH],
                    junkw_t[:], junkx_t[:], start=True, stop=True,
                )

            # ---- prelude: self-similarity blocks (diag -> s_ii) ----
            pre_ps = psp.tile([128, W], F32, tag="chunk", name="pre_ps")
            for t in range(NT):
                nc.tensor.matmul(
                    pre_ps[:, t * 128:(t + 1) * 128], anch(t), anch(t),
                    start=True, stop=True,
                )
            # raw positive segment sums as matmuls: raw3[i, c] = e_i . g_c
            # (g_c = class-sum embedding vectors, 3 extra anchT columns) --
            # keeps the [128, B] raw reductions off the DVE/PSUM critical path
            for t in range(NT):
                nc.tensor.matmul(
                    pre_ps[:, 256 + t * C:256 + (t + 1) * C], anch(t),
                    anch_t[:, APC:APC + C], start=True, stop=True,
                )
            for t in range(NT):
                nc.vector.tensor_mul(eyemul[:], pre_ps[:, t * 128:(t + 1) * 128], eye_t[:])
                nc.vector.reduce_sum(sdiag[t][:], eyemul[:], axis=AX.X)
                nc.vector.tensor_copy(out=raw3[t][:], in_=pre_ps[:, 256 + t * C:256 + (t + 1) * C])
                nc.scalar.activation(
                    selfe[t][:], sdiag[t][:], AF.Exp,
                    bias=ninvt_t[:, t:t + 1], scale=invt_t[:, t:t + 1],
                )

            by_chunk = {}
            for k, (s, e) in enumerate(subs):
                by_chunk.setdefault(s // W, []).append((s, e, k))

            scrNK = [smp.tile([128, NK], F32, tag=f"scrNK{t}", name=f"scrNK{t}") for t in range(NT)]
            scrC = [smp.tile([128, C], F32, tag=f"scrC{t}", name=f"scrC{t}") for t in range(NT)]

            def epi_early(t):
                """olin = coefv*invt*(1 - pos): depends only on prelude
                outputs (raw3/sdiag), so it runs during the exp stream."""
                own_r = smp.tile([128, 1], F32, tag=f"ownr{t}", name=f"ownr{t}")
                pos = smp.tile([128, 1], F32, tag=f"pos{t}", name=f"pos{t}")
                w1 = smp.tile([128, 1], F32, tag=f"w1{t}", name=f"w1{t}")
                p1 = smp.tile([128, 1], F32, tag=f"p1{t}", name=f"p1{t}")
                nc.vector.tensor_mul(scrC[t][:], raw3[t][:], oneh_t[:, t * C:(t + 1) * C])
                nc.vector.reduce_sum(own_r[:], scrC[t][:], axis=AX.X)
                nc.vector.scalar_tensor_tensor(
                    out=pos[:], in0=own_r[:], scalar=sdiag[t][:], in1=invpc_t[:, t:t + 1],
                    op0=ALU.subtract, op1=ALU.mult,
                )
                nc.vector.scalar_tensor_tensor(
                    out=w1[:], in0=pos[:], scalar=-1.0, in1=invt_t[:, t:t + 1],
                    op0=ALU.mult, op1=ALU.mult,
                )
                nc.vector.scalar_tensor_tensor(
                    out=oout_t[:, NT + t:NT + t + 1], in0=w1[:], scalar=invt_t[:, t:t + 1],
                    in1=coefv_t[:, t:t + 1], op0=ALU.add, op1=ALU.mult,
                )
                return p1

            p1s = {}

            def epilogue(t):
                """den = (bbsum - selfe) + sum_k esum_k * incl_k -- the only
                work that must trail the exp stream."""
                nc.vector.tensor_mul(scrNK[t][:], esum[t][:], incl_t[:, t * NK:(t + 1) * NK])
                nc.vector.reduce_sum(oout_t[:, t:t + 1], scrNK[t][:], axis=AX.X)
                nc.vector.tensor_add(oout_t[:, t:t + 1], oout_t[:, t:t + 1], p1s[t][:])

            def emit_bb(t):
                ps = psp.tile([128, W], F32, tag="chunk", name="bb_ps")
                for q in range(W // CH):
                    nc.tensor.matmul(
                        ps[:, q * CH:(q + 1) * CH], anch(t),
                        emb_t[:, q * CH:(q + 1) * CH],
                        start=True, stop=True,
                    )
                nc.scalar.activation(
                    scr_t[:], ps[:], AF.Exp,
                    bias=ninvt_t[:, t:t + 1], scale=invt_t[:, t:t + 1],
                    accum_out=bbsum[t][:],
                )

            def emit_bank(t, j):
                ps = psp.tile([128, W], F32, tag="chunk", name="bk_ps")
                for q in range(W // CH):
                    nc.tensor.matmul(
                        ps[:, q * CH:(q + 1) * CH], anch(t),
                        bank_ts[j][:, q * CH:(q + 1) * CH],
                        start=True, stop=True,
                    )
                for (s, e, k) in by_chunk[j]:
                    a, b = s - j * W, e - j * W
                    nc.scalar.activation(
                        scr_t[:, a:b], ps[:, a:b], AF.Exp,
                        bias=ninvt_t[:, t:t + 1], scale=invt_t[:, t:t + 1],
                        accum_out=esum[t][:, k:k + 1],
                    )

            # all of t0 (its DVE-only epilogue overlaps t1's stream); t1's
            # first chunk emitted before t0's last so PE never drains
            for t in range(NT):
                p1s[t] = epi_early(t)
            emit_bb(0)
            nc.vector.tensor_sub(p1s[0][:], bbsum[0][:], selfe[0][:])
            for j in range(NBK - 1):
                emit_bank(0, j)
            emit_bb(1)
            nc.vector.tensor_sub(p1s[1][:], bbsum[1][:], selfe[1][:])
            emit_bank(0, NBK - 1)
            epilogue(0)
            for j in range(NBK):
                emit_bank(1, j)
            epilogue(1)

            nc.sync.dma_start(out=oout_d[:], in_=oout_t[:])

    nc.compile()
    return nc


def _per_core_cols(vec, core):
    """[B] host vector -> [128, NT] tile for one core (col t, partition p)."""
    sl = vec[core * APC:(core + 1) * APC]
    return np.ascontiguousarray(sl.reshape(NT, 128).T).astype(np.float32)


def kernel(embeddings, labels, bank_embs, bank_labels, class_temps):
    global LAST_EXEC_TIME_NS
    import ml_dtypes

    emb = np.asarray(embeddings, dtype=np.float32)
    bank = np.asarray(bank_embs, dtype=np.float32)
    lab = np.asarray(labels).astype(np.int64).ravel()
    blab = np.asarray(bank_labels).astype(np.int64).ravel()
    ct = np.asarray(class_temps, dtype=np.float32).ravel()

    bord = np.argsort(lab, kind="stable")
    slab = lab[bord]
    mord = np.argsort(blab, kind="stable")
    cnt = np.bincount(lab, minlength=C)
    mcnt = np.bincount(blab, minlength=C)
    bb_b1, bb_b2 = int(cnt[0]), int(cnt[0] + cnt[1])
    mk_b1, mk_b2 = int(mcnt[0]), int(mcnt[0] + mcnt[1])

    embT = np.ascontiguousarray(emb[bord].T)      # [D, B]
    bankT = np.ascontiguousarray(bank[mord].T)    # [D, M]
    if MM_MODE == "bf16":
        embT = embT.astype(ml_dtypes.bfloat16)
        bankT = bankT.astype(ml_dtypes.bfloat16)

    temps = ct[slab]
    inv_t = (1.0 / temps).astype(np.float32)
    pos_cnt = cnt[slab] - 1
    invpc = (1.0 / np.maximum(pos_cnt, 1)).astype(np.float32)
    validf = (pos_cnt > 0).astype(np.float32)
    coefv = (BASE_TEMP / temps).astype(np.float32) * validf
    oneh = np.eye(C, dtype=np.float32)[slab]      # [B, 3]
    n_valid = int((pos_cnt > 0).sum())

    nc = _build(bb_b1, bb_b2, mk_b1, mk_b2, MM_MODE)

    subs, _, _ = _bank_subranges(mk_b1, mk_b2)
    NK = len(subs)
    sub_cls = np.array([0 if s < mk_b1 else (1 if s < mk_b2 else 2) for s, _ in subs])
    # incl[anchor, k] = 1 where subrange class != anchor class
    incl_full = (sub_cls[None, :] != slab[:, None]).astype(np.float32)  # [B, NK]
    eye128 = np.eye(128, dtype=np.float32)

    # per-class embedding-sum vectors for the positives matmul
    gT = np.stack([emb[bord][slab == c].sum(axis=0) for c in range(C)], axis=1)
    gT = np.ascontiguousarray(gT).astype(embT.dtype)

    in_maps = []
    for core in range(NCORES):
        asl = slice(core * APC, (core + 1) * APC)
        oh = oneh[asl].reshape(NT, 128, C).transpose(1, 0, 2).reshape(128, NT * C)
        ic = incl_full[asl].reshape(NT, 128, NK).transpose(1, 0, 2).reshape(128, NT * NK)
        vecs = np.concatenate([
            _per_core_cols(inv_t, core),
            _per_core_cols(-inv_t, core),
            _per_core_cols(invpc, core),
            _per_core_cols(coefv, core),
            oh.astype(np.float32),
            ic.astype(np.float32),
            eye128,
        ], axis=1)
        in_maps.append({
            "embT": embT,
            "anchT": np.ascontiguousarray(np.concatenate([embT[:, asl], gT], axis=1)),
            "bankT": bankT,
            "vecs": np.ascontiguousarray(vecs),
        })

    trace = os.environ.get("SUPCON_TRACE", "0") == "1"
    if trace:
        trace = _install_trace_shim()
    res = run_bass_kernel_spmd(nc, in_maps, core_ids=list(range(NCORES)), trace=trace)
    LAST_EXEC_TIME_NS = res.exec_time_ns

    # loss_i = coef_i * log(den_i) + lin_i ; device produced den/lin,
    # host finishes the 2048 scalar logs + masked mean
    loss_sum = np.float64(0.0)
    for core in range(NCORES):
        oo = np.asarray(res.results[core]["oout"], dtype=np.float64)    # [128, 2*NT]
        den, lin = oo[:, :NT], oo[:, NT:]
        cf = _per_core_cols(coefv, core).astype(np.float64)
        loss_sum += (cf * np.log(den) + lin).sum()
    return np.float32(loss_sum / max(n_valid, 1))



# revision 5
# speedup vs baseline: 1.1524x; 1.1524x over previous
"""ClassBalancedSupConLoss on 8 TRN2 NeuronCores (Bass/Tile) — v2.

Key change vs v1: CLASS-SKIP.  Anchors are re-permuted into class-pure
128-row tiles (leftovers form 1-2 mixed tiles).  A pure class-c tile's
denominator needs bank columns of classes != c only, so each core's
bank stream is a host-PACKED dense array of just those columns — the
own-class ~1/3 of the bank (previously exp'd and then subtracted) is
never computed.  bb (batch) columns are always fully included (positives
are part of the reference denominator), so bank+bb merge into a single
running accumulation per tile: one accum_out per PSUM block, summed on
the host.  The self term exp(invt*(s_ii-1)) is subtracted on the HOST
from the device-computed s_ii (spline-vs-np.exp difference ~2ULP is
negligible against the 2e-2 gate).

SPMD uniformity: one program for all 8 cores; all per-core variation is
in the packed DATA (which bank columns, per-call inclusion masks and
zero-pad dummy counts live host-side).  Program constants (segment
sizes, call cuts) come from the label histograms, baked at compile time.

Layout per core (wslot = weight slot: 0,1 = own tiles, 2.. = mixed
foreign tiles shared by all cores):
  stream = [slot0: bank KB | bb 2048]   (one cut-group: pure tile)
           [slot1: group m_x | group m_y | bb 2048]  (cuts at groups)
           [FS: ceil(m_cmin/8) per mixed tile]       (foreign share)
Calls = PSUM blocks (<=2048 cols), new block at every cut-group/wslot
change.  Host reduction: den_i = sum over included calls of
(accum - ndum*exp(-invt_i)) - exp(invt_i*(sdiag_i-1)).
"""

import os
import numpy as np

import concourse.bass as bass  # noqa: F401
from concourse import bacc
import concourse.mybir as mybir
import concourse.tile as tile
from concourse.bass_utils import run_bass_kernel_spmd

B, D, M, C = 2048, 128, 16384, 3
NCORES = 8
NTILES = B // 128          # 16 anchor tiles of 128
CH = 512                   # matmul free chunk (one PSUM bank)
W = 2048                   # PSUM block (4 banks) = one ACT call
BASE_TEMP = 0.07

F32 = mybir.dt.float32
BF16 = mybir.dt.bfloat16
AF = mybir.ActivationFunctionType
ALU = mybir.AluOpType
AX = mybir.AxisListType

LAST_EXEC_TIME_NS = None   # set by kernel() when SUPCON_TRACE=1


def _install_trace_shim():
    """Register the NTFF profile hook that this image's antenv lacks."""
    import sys
    import types
    import ctypes
    import contextlib

    try:
        from antenv.axon_hooks import get_axon_ntff_profile_hook  # noqa: F401
        return True
    except ImportError:
        pass

    so_path = "/opt/axon/libaxon_pjrt.so"
    if not os.path.exists(so_path):
        return False
    lib = ctypes.CDLL(so_path)
    if not hasattr(lib, "axon_start_nrt_profile"):
        return False
    lib.axon_start_nrt_profile.argtypes = [
        ctypes.POINTER(ctypes.c_int64),
        ctypes.c_size_t,
    ]
    lib.axon_start_nrt_profile.restype = ctypes.c_int64
    lib.axon_stop_nrt_profile.argtypes = [ctypes.c_char_p]
    lib.axon_stop_nrt_profile.restype = ctypes.c_int64

    @contextlib.contextmanager
    def _hook(output_dir, device_ids):
        import jax

        jax.devices()
        if device_ids:
            ids = (ctypes.c_int64 * len(device_ids))(*device_ids)
            rc = lib.axon_start_nrt_profile(ids, len(device_ids))
        else:
            rc = lib.axon_start_nrt_profile(None, 0)
        if rc != 0:
            raise RuntimeError(f"axon_start_nrt_profile rc={rc}")
        try:
            yield
        finally:
            n = lib.axon_stop_nrt_profile(str(output_dir).encode())
            print(f"profile: {n} file(s) written to {output_dir}", file=sys.stderr)

    _state = {"hook": _hook}
    mod = types.ModuleType("antenv.axon_hooks")
    mod.get_axon_ntff_profile_hook = lambda: _state["hook"]
    mod.set_axon_ntff_profile_hook = lambda h: _state.update(hook=h)
    sys.modules["antenv.axon_hooks"] = mod
    import antenv

    antenv.axon_hooks = mod

    import concourse.bass_utils as bu

    bu.upload_artifacts = lambda tmpdir: tmpdir
    return True


# ----------------------------------------------------------------------
# Host planning
# ----------------------------------------------------------------------

def _make_plan(lab, blab):
    """Compile-time plan from the label histograms (baked into the
    program; identical for all cores)."""
    cnt = np.bincount(lab, minlength=C)
    by_class = [np.where(lab == c)[0] for c in range(C)]
    fb = [(int(cnt[c]) // 128) * 128 for c in range(C)]
    pure_idx = np.concatenate([by_class[c][: fb[c]] for c in range(C)])
    left_idx = np.concatenate([by_class[c][fb[c]:] for c in range(C)])
    bord = np.concatenate([pure_idx, left_idx]).astype(np.int64)
    slab = lab[bord]

    tile_classes = [
        sorted(set(slab[t * 128:(t + 1) * 128].tolist())) for t in range(NTILES)
    ]
    mixed_ids = [t for t in range(NTILES) if len(tile_classes[t]) > 1]
    nm = len(mixed_ids)

    mord = np.argsort(blab, kind="stable").astype(np.int64)
    m = np.bincount(blab, minlength=C).astype(np.int64)
    seg = [0, int(m[0]), int(m[0] + m[1]), M]

    # position p (0..15) -> tile id; mixed tiles must land on odd
    # positions (slot1) of the last cores.
    pure_ids = [t for t in range(NTILES) if t not in mixed_ids]
    tile_of_pos = [None] * NTILES
    mixed_pos = [NTILES - 1 - 2 * i for i in range(nm)]  # 15, 13
    for i, p in enumerate(mixed_pos):
        tile_of_pos[p] = mixed_ids[nm - 1 - i]
    it = iter(pure_ids)
    for p in range(NTILES):
        if tile_of_pos[p] is None:
            tile_of_pos[p] = next(it)

    cmin = int(np.argmin(m))
    gclasses = [c for c in range(C) if c != cmin]        # slot1 group classes
    gsizes = [int(m[c]) for c in gclasses]
    KB = sum(gsizes)                                     # = M - m[cmin]

    # FS: per mixed tile, the cmin-class segment striped over 8 cores
    fs_per = int(-(-int(m[cmin]) // NCORES)) if nm else 0
    fs_runs = [
        {"wslot": 2 + i, "cls": cmin, "per_core": fs_per, "total": int(m[cmin])}
        for i in range(nm)
    ]
    F0 = fs_per * nm

    return {
        "bord": bord, "mord": mord, "slab": slab, "m": m, "seg": seg,
        "cnt": cnt, "tile_of_pos": tile_of_pos, "mixed_ids": mixed_ids,
        "nm": nm, "cmin": cmin, "gclasses": gclasses, "gsizes": gsizes,
        "KB": KB, "fs_runs": fs_runs, "F0": F0,
    }


def _make_stream(plan):
    """The uniform per-core column stream: list of segments
    (region, src_off, length, wslot, cutgroup).  region in
    {'A','B','F','E'} (bankA, bankB, bankF SBUF tiles, emb)."""
    KB = plan["KB"]
    segs = []
    segs.append(("A", 0, KB, 0, "s0"))
    segs.append(("E", 0, B, 0, "s0"))          # slot0 bank+bb share a group
    off = 0
    for c, g in zip(plan["gclasses"], plan["gsizes"]):
        segs.append(("B", off, g, 1, f"s1g{c}"))
        off += g
    segs.append(("E", 0, B, 1, "s1e"))
    foff = 0
    for r in plan["fs_runs"]:
        segs.append(("F", foff, r["per_core"], r["wslot"], f"fs{r['wslot']}"))
        foff += r["per_core"]
    return segs


def _make_blocks(segs):
    """Blocks = ACT calls.  New block at every cutgroup change; within a
    group, 2048-col blocks.  Each block: list of chunks
    (region, src_off, width, wslot) with width<=512, plus call meta."""
    blocks = []
    cur = None

    def flush():
        nonlocal cur
        if cur and cur["width"] > 0:
            blocks.append(cur)
        cur = None

    for (reg, soff, length, ws, grp) in segs:
        pos = 0
        while pos < length:
            if cur is not None and (cur["grp"] != grp or cur["width"] >= W):
                flush()
            if cur is None:
                cur = {"grp": grp, "wslot": ws, "width": 0, "chunks": []}
            take = min(length - pos, W - cur["width"])
            # split into <=512 matmul chunks ALIGNED to PSUM banks: a
            # single matmul output cannot cross a 512-col PSUM bank edge
            cpos = 0
            while cpos < take:
                ppos = cur["width"] + cpos
                cw = min(CH - (ppos % CH), take - cpos)
                cur["chunks"].append((reg, soff + pos + cpos, cw, ws))
                cpos += cw
            cur["width"] += take
            pos += take
            if cur["width"] >= W:
                flush()
    flush()
    return blocks


# ----------------------------------------------------------------------
# Device program
# ----------------------------------------------------------------------

def _build(plan, blocks):
    import ml_dtypes  # noqa: F401

    KB, F0, nm = plan["KB"], plan["F0"], plan["nm"]
    NW = 2 + nm
    NCALLS = len(blocks)
    NOUT = NCALLS + 2 + 6   # accums | sdiag x2 | raw3 x2

    nc = bacc.Bacc()
    embT_d = nc.declare_dram_parameter("embT", [D, B], BF16, isOutput=False)
    anchT_d = nc.declare_dram_parameter("anchT", [D, 128 * NW + C], BF16,
                                        isOutput=False)
    bankA_d = nc.declare_dram_parameter("bankA", [D, KB], BF16, isOutput=False)
    bankB_d = nc.declare_dram_parameter("bankB", [D, KB], BF16, isOutput=False)
    if F0:
        bankF_d = nc.declare_dram_parameter("bankF", [D, F0], BF16,
                                            isOutput=False)
    # vecs: [invt x NW | ninvt x NW | eye128]
    NV = 2 * NW + 128
    vecs_d = nc.declare_dram_parameter("vecs", [128, NV], F32, isOutput=False)
    oout_d = nc.declare_dram_parameter("oout", [128, NOUT], F32, isOutput=True)

    with tile.TileContext(nc) as tc:
        with (
            tc.tile_pool(name="big", bufs=1) as bigp,
            tc.tile_pool(name="sm", bufs=1) as smp,
            tc.tile_pool(name="ps", bufs=2, space="PSUM") as psp,
        ):
            anch_t = bigp.tile([D, 128 * NW + C], BF16, tag="anchT")
            vecs_t = smp.tile([128, NV], F32, tag="vecs")
            junkw_t = bigp.tile([128, 128], BF16, tag="junkw")
            junkx_t = bigp.tile([128, CH], BF16, tag="junkx")
            emb_t = bigp.tile([D, B], BF16, tag="embT")
            bankA_t = bigp.tile([D, KB], BF16, tag="bankA")
            bankB_t = bigp.tile([D, KB], BF16, tag="bankB")
            if F0:
                bankF_t = bigp.tile([D, F0], BF16, tag="bankF", name="bankF_t")
            else:
                bankF_t = None
            scr_t = smp.tile([128, W], F32, tag="scr")
            oout_t = smp.tile([128, NOUT], F32, tag="oout")
            eyemul = smp.tile([128, 128], F32, tag="eyemul")
            warm = smp.tile([128, 1], F32, tag="warm")

            invt_t = vecs_t[:, 0:NW]
            ninvt_t = vecs_t[:, NW:2 * NW]
            eye_t = vecs_t[:, 2 * NW:2 * NW + 128]

            regions = {"A": bankA_t, "B": bankB_t, "F": bankF_t, "E": emb_t}

            def anch(ws):
                return anch_t[:, ws * 128:(ws + 1) * 128]

            # ---- DMA (2 HWDGE queues, consumption order) ----
            qs = [nc.sync, nc.scalar]
            qi = [0]

            def dma(out_ap, in_ap):
                qs[qi[0] % 2].dma_start(out=out_ap, in_=in_ap)
                qi[0] += 1

            dma(vecs_t[:], vecs_d[:])
            dma(anch_t[:], anchT_d[:])

            def dma_pieces(t, d, total, first_small=False):
                pos = 0
                if first_small and total > CH:
                    dma(t[:, 0:CH], d[:, 0:CH])
                    pos = CH
                while pos < total:
                    take = min(W, total - pos)
                    dma(t[:, pos:pos + take], d[:, pos:pos + take])
                    pos += take

            dma_pieces(bankA_t, bankA_d, KB, first_small=True)
            dma_pieces(emb_t, embT_d, B)
            dma_pieces(bankB_t, bankB_d, KB)
            if F0:
                dma_pieces(bankF_t, bankF_d, F0)

            # ---- ACT table-load warmup (t ~ 0, no DMA dep) ----
            nc.vector.memset(junkw_t[:], 0.0)
            nc.vector.memset(junkx_t[:], 0.0)
            nc.scalar.activation(warm[:], junkw_t[:, 0:1], AF.Exp,
                                 bias=0.0, scale=0.0)

            # ---- PE/HAM warmup on garbage operands ----
            warm_ps = psp.tile([128, W], F32, tag="chunk", name="warm_ps")
            for w in range(8):
                nc.tensor.matmul(
                    warm_ps[:, (w % 4) * CH:((w % 4) + 1) * CH],
                    junkw_t[:], junkx_t[:], start=True, stop=True,
                )

            # ---- prelude: self-sim diags + raw3 (class-sum dots) ----
            pre_ps = psp.tile([128, W], F32, tag="chunk", name="pre_ps")
            for t in range(2):
                nc.tensor.matmul(
                    pre_ps[:, t * 128:(t + 1) * 128], anch(t), anch(t),
                    start=True, stop=True,
                )
                nc.tensor.matmul(
                    pre_ps[:, 256 + t * C:256 + (t + 1) * C], anch(t),
                    anch_t[:, 128 * NW:128 * NW + C], start=True, stop=True,
                )
            for t in range(2):
                nc.vector.tensor_mul(eyemul[:], pre_ps[:, t * 128:(t + 1) * 128],
                                     eye_t[:])
                nc.vector.reduce_sum(oout_t[:, NCALLS + t:NCALLS + t + 1],
                                     eyemul[:], axis=AX.X)
                nc.vector.tensor_copy(
                    out=oout_t[:, NCALLS + 2 + t * C:NCALLS + 2 + (t + 1) * C],
                    in_=pre_ps[:, 256 + t * C:256 + (t + 1) * C])

            # ---- main stream: blocks of matmul chunks + one EXP call ----
            for j, blk in enumerate(blocks):
                ps = psp.tile([128, W], F32, tag="chunk", name=f"blk{j}")
                pos = 0
                for (reg, soff, cw, ws) in blk["chunks"]:
                    nc.tensor.matmul(
                        ps[:, pos:pos + cw], anch(ws),
                        regions[reg][:, soff:soff + cw],
                        start=True, stop=True,
                    )
                    pos += cw
                wsl = blk["wslot"]
                nc.scalar.activation(
                    scr_t[:, 0:blk["width"]], ps[:, 0:blk["width"]], AF.Exp,
                    bias=ninvt_t[:, wsl:wsl + 1], scale=invt_t[:, wsl:wsl + 1],
                    accum_out=oout_t[:, j:j + 1],
                )

            nc.sync.dma_start(out=oout_d[:], in_=oout_t[:])

    nc.compile()
    return nc


# ----------------------------------------------------------------------
# Host packing + reduction
# ----------------------------------------------------------------------

def kernel(embeddings, labels, bank_embs, bank_labels, class_temps):
    global LAST_EXEC_TIME_NS
    import ml_dtypes

    emb = np.asarray(embeddings, dtype=np.float32)
    bank = np.asarray(bank_embs, dtype=np.float32)
    lab = np.asarray(labels).astype(np.int64).ravel()
    blab = np.asarray(bank_labels).astype(np.int64).ravel()
    ct = np.asarray(class_temps, dtype=np.float32).ravel()

    plan = _make_plan(lab, blab)
    segs = _make_stream(plan)
    blocks = _make_blocks(segs)
    NCALLS = len(blocks)
    nm, KB, F0 = plan["nm"], plan["KB"], plan["F0"]
    NW = 2 + nm
    bord, mord, slab = plan["bord"], plan["mord"], plan["slab"]
    seg, cmin = plan["seg"], plan["cmin"]
    tile_of_pos = plan["tile_of_pos"]

    embT = np.ascontiguousarray(emb[bord].T).astype(ml_dtypes.bfloat16)  # [D,B]
    bankT = np.ascontiguousarray(bank[mord].T).astype(ml_dtypes.bfloat16)
    smlab = blab[mord]

    temps = ct[slab]
    inv_t = (1.0 / temps).astype(np.float32)             # [B] sorted order
    cnt = plan["cnt"]
    pos_cnt = cnt[slab] - 1
    n_valid = int((pos_cnt > 0).sum())

    # class-sum embedding vectors (from the same bf16-rounded data)
    gT = np.stack(
        [emb[bord][slab == c].sum(axis=0) for c in range(C)], axis=1
    ).astype(ml_dtypes.bfloat16)                          # [D, 3]

    # per-class bank column index lists (positions in mord order)
    cls_cols = [np.arange(seg[c], seg[c + 1]) for c in range(C)]

    def pure_cols(c):
        return np.concatenate([cls_cols[cc] for cc in range(C) if cc != c])

    # pack one bank slot: returns (bf16 [D, width], tag [width])
    def pack(cols, width):
        out = np.zeros((D, width), dtype=ml_dtypes.bfloat16)
        tags = np.full(width, -2, dtype=np.int64)
        k = len(cols)
        out[:, :k] = bankT[:, cols]
        tags[:k] = smlab[cols]
        return out, tags

    # mixed-tile home slot: group-aligned packing
    def mixed_home():
        outs, tags = [], []
        for c, g in zip(plan["gclasses"], plan["gsizes"]):
            o, t = pack(cls_cols[c], g)
            outs.append(o)
            tags.append(t)
        return np.concatenate(outs, axis=1), np.concatenate(tags)

    in_maps = []
    tagsA, tagsB, tagsF = [], [], []
    core_tiles = []
    for core in range(NCORES):
        t0, t1 = tile_of_pos[2 * core], tile_of_pos[2 * core + 1]
        core_tiles.append((t0, t1))
        wslot_tiles = [t0, t1] + plan["mixed_ids"]

        def tclass(t):
            s = set(slab[t * 128:(t + 1) * 128].tolist())
            return next(iter(s)) if len(s) == 1 else None

        a_cols = pure_cols(tclass(t0))
        bA, tgA = pack(a_cols, KB)
        if t1 in plan["mixed_ids"]:
            bB, tgB = mixed_home()
        else:
            bB, tgB = pack(pure_cols(tclass(t1)), KB)
        tagsA.append(tgA)
        tagsB.append(tgB)

        fs_parts, fs_tags = [], []
        for r in plan["fs_runs"]:
            lo = core * r["per_core"]
            hi = min(lo + r["per_core"], r["total"])
            cols = cls_cols[r["cls"]][lo:hi] if hi > lo else np.array([], int)
            o, t = pack(cols, r["per_core"])
            fs_parts.append(o)
            fs_tags.append(t)
        tagsF.append(np.concatenate(fs_tags) if fs_parts else
                     np.zeros(0, dtype=np.int64))

        anchT = np.zeros((D, 128 * NW + C), dtype=ml_dtypes.bfloat16)
        for w, t in enumerate(wslot_tiles):
            anchT[:, w * 128:(w + 1) * 128] = embT[:, t * 128:(t + 1) * 128]
        anchT[:, 128 * NW:] = gT

        vecs = np.zeros((128, 2 * NW + 128), dtype=np.float32)
        for w, t in enumerate(wslot_tiles):
            vecs[:, w] = inv_t[t * 128:(t + 1) * 128]
            vecs[:, NW + w] = -inv_t[t * 128:(t + 1) * 128]
        vecs[:, 2 * NW:] = np.eye(128, dtype=np.float32)

        im = {
            "embT": embT,
            "anchT": np.ascontiguousarray(anchT),
            "bankA": np.ascontiguousarray(bA),
            "bankB": np.ascontiguousarray(bB),
            "vecs": np.ascontiguousarray(vecs),
        }
        if F0:
            im["bankF"] = np.ascontiguousarray(np.concatenate(fs_parts, axis=1))
        in_maps.append(im)

    nc = _build(plan, blocks)

    trace = os.environ.get("SUPCON_TRACE", "0") == "1"
    if trace:
        trace = _install_trace_shim()
    res = run_bass_kernel_spmd(nc, in_maps, core_ids=list(range(NCORES)),
                               trace=trace)
    LAST_EXEC_TIME_NS = res.exec_time_ns

    # ---- host reduction ----
    # per-core per-call class sets + dummy counts from the tag arrays
    region_tags = {"A": tagsA, "B": tagsB, "F": tagsF}
    den = np.zeros(B, dtype=np.float64)           # sorted-anchor order
    sdiag = np.zeros(B, dtype=np.float64)
    raw3 = np.zeros((B, C), dtype=np.float64)
    einv = np.exp(-inv_t.astype(np.float64))      # exp(-invt_i) per anchor

    accs = []
    for core in range(NCORES):
        oo = np.asarray(res.results[core]["oout"], dtype=np.float64)
        accs.append(oo)
        t0, t1 = core_tiles[core]
        for t, slot in ((t0, 0), (t1, 1)):
            rows = slice(t * 128, (t + 1) * 128)
            sdiag[rows] = oo[:, NCALLS + slot]
            raw3[rows] = oo[:, NCALLS + 2 + slot * C:NCALLS + 2 + (slot + 1) * C]

    for core in range(NCORES):
        oo = accs[core]
        t0, t1 = core_tiles[core]
        wslot_tiles = [t0, t1] + plan["mixed_ids"]
        for j, blk in enumerate(blocks):
            t = wslot_tiles[blk["wslot"]]
            rows = slice(t * 128, (t + 1) * 128)
            lt = slab[rows]
            # gather this call's bank tags (bb chunks have no tags)
            tags = []
            for (reg, soff, cw, _ws) in blk["chunks"]:
                if reg != "E":
                    tags.append(region_tags[reg][core][soff:soff + cw])
            if tags:
                tags = np.concatenate(tags)
                ndum = int((tags == -2).sum())
                cls_set = set(tags[tags >= 0].tolist())
            else:
                ndum, cls_set = 0, set()
            inc = ~np.isin(lt, list(cls_set)) if cls_set else \
                np.ones(128, dtype=bool)
            contrib = oo[:, j] - ndum * einv[rows]
            den[rows] += np.where(inc, contrib, 0.0)

    den -= np.exp(inv_t.astype(np.float64) * (sdiag - 1.0))

    own_raw = raw3[np.arange(B), slab]
    pos_mean_raw = (own_raw - sdiag) / np.maximum(pos_cnt, 1)
    invt64 = inv_t.astype(np.float64)
    coef = (BASE_TEMP / temps).astype(np.float64)
    # loss_i = coef * (invt*(1 - pos_mean_raw) + log(den))
    loss_i = coef * (invt64 * (1.0 - pos_mean_raw) + np.log(den))
    valid = pos_cnt > 0
    loss = np.where(valid, loss_i, 0.0).sum() / max(n_valid, 1)
    return np.float32(loss)
